# revision 1
# baseline (speedup 1.0000x reference)
"""Trainium2 Bass kernel: cached causal self-attention (dense transformer block).

Full module: y = CausalAttn(x; Wq, Wk, Wv) @ Wo.T + bo with
  B=4, S=2048, E=2048, H=16 heads, Dh=128, fp32 inputs.

Distribution: 8-way tensor parallel over heads (2 heads per NeuronCore).
Each core computes Q/K/V projections for its 2 heads, causal-softmax
attention, and a partial output projection; the host sums the 8 partials
and adds the bias.

All matmul operands are bf16 (PSUM accumulation stays fp32): same PE
streaming rate as float32r but Fast-Weight-Load halves the LDWEIGHTS
cost, SBUF/DMA traffic halves, and power throttling drops.  End-to-end
rel err ~5e-3 (validated offline), well inside the 2e-2 gate.

Layout: x pre-transposed on host (xT [E, B*S]); scores computed
transposed (sT[k, q]) so exp(sT) feeds attn@V directly with no on-chip
transpose.  Softmax denominators come from a ones-vector matmul
accumulated in PSUM.

Schedule (the point of this version): the two heads' attention k-tile
loops are interleaved so the PE never waits on the scalar engine's exp
 -- per k-tile step the PE runs s0,s1,[proj filler],av0,dn0,av1,dn1
while ACT runs exp0,exp1 of the previous step's scores.  The output
projection accumulates BOTH heads into one PSUM bank (ctx is
pre-normalized by 1/den), turning the eviction into a plain copy, and
its matmul pairs are spread through the attention steps and the next
batch's QKV phase as PE filler work.  1/den is produced per (g,h) by a
DMA round-trip through DRAM (transpose to [128,4], DVE reciprocal,
transpose back, partition-broadcast) entirely on the idle gpsimd/DMA
path.
"""

import math

import ml_dtypes
import numpy as np

import concourse.bacc as bacc
import concourse.mybir as mybir
import concourse.tile as tile
from concourse.bass_utils import run_bass_kernel_spmd

F32 = mybir.dt.float32
BF16 = mybir.dt.bfloat16
AF = mybir.ActivationFunctionType
ALU = mybir.AluOpType

NEG = -1.0e30

# Full-problem constants
EMB = 2048
N_HEADS = 16
HEAD_DIM = 128
B_FULL = 4
S_FULL = 2048
N_CORES = 8
HPC = N_HEADS // N_CORES  # heads per core = 2


def build(B=B_FULL, S=S_FULL, E=EMB, hpc=HPC, DH=HEAD_DIM, CH=512):
    """Build the per-core Bass program (same program on all 8 cores)."""
    assert hpc == 2
    SB = B * S
    DHC = hpc * DH          # per-core head dims (256)
    NE = E // 128           # e-tiles (contraction tiles)
    NEH = NE // 2
    NCH = S // CH           # 512-wide chunks per sequence
    KPC = CH // 128         # k-tiles per chunk (4)
    NST = S // 128          # 128-row s-tiles per sequence
    NOC = E // CH           # output chunks
    scale = 1.0 / math.sqrt(DH)

    nc = bacc.Bacc("TRN2", target_bir_lowering=False, debug=False,
                   num_devices=N_CORES)

    xT = nc.dram_tensor("xT", [E, SB], BF16, kind="ExternalInput")
    wqT = nc.dram_tensor("wqT", [E, DHC], BF16, kind="ExternalInput")
    wkT = nc.dram_tensor("wkT", [E, DHC], BF16, kind="ExternalInput")
    wvT = nc.dram_tensor("wvT", [E, DHC], BF16, kind="ExternalInput")
    woT = nc.dram_tensor("woT", [DHC, E], BF16, kind="ExternalInput")
    masks = nc.dram_tensor("masks", [128, 128], BF16, kind="ExternalInput")
    ones = nc.dram_tensor("ones", [128, 1], BF16, kind="ExternalInput")
    y = nc.dram_tensor("y", [SB, E], BF16, kind="ExternalOutput")

    with tile.TileContext(nc) as tc:
        with (
            tc.tile_pool(name="wpool", bufs=1) as wpool,
            tc.tile_pool(name="xtp", bufs=2) as xtp,
            tc.tile_pool(name="qpool", bufs=1) as qpool,
            tc.tile_pool(name="kvpool", bufs=2) as kvpool,
            tc.tile_pool(name="ctxup", bufs=2) as ctxup,
            tc.tile_pool(name="expp", bufs=4) as expp,
            tc.tile_pool(name="denp", bufs=2) as denp,
            tc.tile_pool(name="dramp", bufs=3, space="DRAM") as dramp,
            tc.tile_pool(name="yp", bufs=4) as yp,
            tc.tile_pool(name="ps_sp", bufs=2, space="PSUM") as ps_sp,
            tc.tile_pool(name="ps_av", bufs=1, space="PSUM") as ps_av,
            tc.tile_pool(name="ps_dn", bufs=1, space="PSUM") as ps_dn,
            tc.tile_pool(name="ps_pj", bufs=2, space="PSUM") as ps_pj,
        ):
            # Resident weights / constants
            wq_sb = wpool.tile([128, NE, DHC], BF16, tag="wq")
            wk_sb = wpool.tile([128, NE, DHC], BF16, tag="wk")
            wv_sb = wpool.tile([128, NE, DHC], BF16, tag="wv")
            wo_sb = wpool.tile([128, hpc, E], BF16, tag="wo")
            xT_r = xT.rearrange("(t p) s -> p t s", p=128)
            wq_r = wqT.rearrange("(t p) d -> p t d", p=128)
            wk_r = wkT.rearrange("(t p) d -> p t d", p=128)
            # interleave weight halves with the first x chunk so the first
            # Q accumulation (wq + x) starts as early as possible
            x0a = xtp.tile([128, NEH, CH], BF16, tag="xta", name="x0a")
            x0b = xtp.tile([128, NEH, CH], BF16, tag="xtb", name="x0b")
            nc.sync.dma_start(wq_sb[:, 0:NEH, :], wq_r[:, 0:NEH, :])
            nc.sync.dma_start(x0a[:], xT_r[:, 0:NEH, 0:CH])
            nc.sync.dma_start(wq_sb[:, NEH:NE, :], wq_r[:, NEH:NE, :])
            nc.sync.dma_start(x0b[:], xT_r[:, NEH:NE, 0:CH])
            nc.sync.dma_start(wk_sb[:, 0:NEH, :], wk_r[:, 0:NEH, :])
            nc.sync.dma_start(wk_sb[:, NEH:NE, :], wk_r[:, NEH:NE, :])
            xpre = ((0, 0), x0a, x0b)
            nc.sync.dma_start(wv_sb[:], wvT.rearrange("(t p) d -> p t d", p=128))
            nc.sync.dma_start(wo_sb[:], woT.rearrange("(h p) e -> p h e", p=128))
            mask_sb = wpool.tile([128, 128], BF16, tag="mask")
            nc.sync.dma_start(mask_sb[:], masks[:, :])
            ones_sb = wpool.tile([128, 1], BF16, tag="ones")
            nc.sync.dma_start(ones_sb[:], ones[:, :])

            evict_parity = [0]

            def emit_proj_tile(pctxn, st, oc, ps0):
                """One output tile [128 q, CH]: both heads accumulated into one
                PSUM bank, plain-copy evict (alternating ACT/DVE), y DMA."""
                p = ps_pj.tile([128, CH], F32, tag="pj")
                o0 = oc * CH
                nc.tensor.matmul(p[:], pctxn[:, 0, st * 128:(st + 1) * 128],
                                 wo_sb[:, 0, o0:o0 + CH], start=True, stop=False)
                nc.tensor.matmul(p[:], pctxn[:, 1, st * 128:(st + 1) * 128],
                                 wo_sb[:, 1, o0:o0 + CH], start=False, stop=True)
                ysb = yp.tile([128, CH], BF16, tag="ysb")
                if evict_parity[0] % 3 == 0:
                    nc.scalar.copy(ysb[:], p[:])
                else:
                    nc.vector.tensor_copy(ysb[:], p[:])
                evict_parity[0] += 1
                nc.gpsimd.dma_start(
                    y[ps0 + st * 128:ps0 + (st + 1) * 128, o0:o0 + CH], ysb[:])

            # pending proj work from the previous batch's last chunk:
            # list of (ctxn_tile, st, ps0) emitted as filler during phase A
            pending = []

            for b in range(B):
                s0 = b * S
                qT = qpool.tile([128, hpc, S], BF16, tag="qT")
                ctxTn = qpool.tile([128, hpc, S], BF16, tag="ctxn")
                kT = kvpool.tile([128, hpc, S], BF16, tag="kT")
                v_sb = kvpool.tile([128, NST, DHC], BF16, tag="v")

                # ---------------- Phase A: Q/K/V projections -------------
                fillers = list(pending)
                pending = []
                fi = 0
                n_groups = NCH * (2 * hpc + KPC)
                gi = 0

                def maybe_fill_a():
                    nonlocal fi, gi
                    gi += 1
                    gd, nd = gi - 4, n_groups - 4
                    while fi < len(fillers) and gd >= 1 and fi + 1 <= (
                            len(fillers) * gd + nd - 1) // nd:
                        pctxn, st, ps0, oc = fillers[fi]
                        emit_proj_tile(pctxn, st, oc, ps0)
                        fi += 1

                for ch in range(NCH):
                    c0 = ch * CH
                    if xpre is not None and xpre[0] == (b, ch):
                        xta, xtb = xpre[1], xpre[2]
                    else:
                        xta = xtp.tile([128, NEH, CH], BF16, tag="xta")
                        nc.sync.dma_start(xta[:],
                                          xT_r[:, 0:NEH, s0 + c0:s0 + c0 + CH])
                        xtb = xtp.tile([128, NEH, CH], BF16, tag="xtb")
                        nc.sync.dma_start(xtb[:],
                                          xT_r[:, NEH:NE, s0 + c0:s0 + c0 + CH])
                    if ch + 1 < NCH or b + 1 < B:
                        nb_, nch = (b, ch + 1) if ch + 1 < NCH else (b + 1, 0)
                        n0 = nb_ * S + nch * CH
                        xna = xtp.tile([128, NEH, CH], BF16, tag="xta",
                                       name="xna")
                        nc.sync.dma_start(xna[:], xT_r[:, 0:NEH, n0:n0 + CH])
                        xnb = xtp.tile([128, NEH, CH], BF16, tag="xtb",
                                       name="xnb")
                        nc.sync.dma_start(xnb[:], xT_r[:, NEH:NE, n0:n0 + CH])
                        xpre = ((nb_, nch), xna, xnb)
                    else:
                        xpre = None

                    def xslice(et, lo=None, hi=None):
                        t = xta if et < NEH else xtb
                        e = et if et < NEH else et - NEH
                        if lo is None:
                            return t[:, e, :]
                        return t[:, e, lo:hi]

                    for h in range(hpc):
                        qp = ps_pj.tile([128, CH], F32, tag="pj")
                        for et in range(NE):
                            nc.tensor.matmul(
                                qp[:], wq_sb[:, et, h * DH:(h + 1) * DH],
                                xslice(et),
                                start=(et == 0), stop=(et == NE - 1))
                        nc.scalar.activation(qT[:, h, c0:c0 + CH], qp[:],
                                             AF.Identity, scale=scale)
                        maybe_fill_a()
                        kp = ps_pj.tile([128, CH], F32, tag="pj")
                        for et in range(NE):
                            nc.tensor.matmul(
                                kp[:], wk_sb[:, et, h * DH:(h + 1) * DH],
                                xslice(et),
                                start=(et == 0), stop=(et == NE - 1))
                        nc.scalar.activation(kT[:, h, c0:c0 + CH], kp[:],
                                             AF.Identity)
                        maybe_fill_a()
                    for st in range(KPC):
                        vp = ps_pj.tile([128, DHC], F32, tag="pj")
                        for et in range(NE):
                            nc.tensor.matmul(
                                vp[:], xslice(et, st * 128, (st + 1) * 128),
                                wv_sb[:, et, :],
                                start=(et == 0), stop=(et == NE - 1))
                        nc.scalar.activation(v_sb[:, ch * KPC + st, :], vp[:],
                                             AF.Identity)
                        maybe_fill_a()
                # any leftover fillers
                while fi < len(fillers):
                    pctxn, st, ps0, oc = fillers[fi]
                    emit_proj_tile(pctxn, st, oc, ps0)
                    fi += 1

                # ------- Phase B: attention, heads interleaved ----------
                for g in range(NCH):
                    nk = KPC * (g + 1)
                    # proj fillers for chunk g-1 of this batch
                    gfill = []
                    if g > 0:
                        for st in range((g - 1) * KPC, g * KPC):
                            for oc in range(NOC):
                                gfill.append((st, oc))
                    gfi = 0
                    avp = [ps_av.tile([128, CH], F32, tag=f"av{h}",
                                    name=f"av{h}")
                           for h in range(hpc)]
                    dnp = [ps_dn.tile([1, CH], F32, tag=f"dn{h}",
                                      name=f"dn{h}")
                           for h in range(hpc)]
                    for kt in range(nk):
                        j = kt - (nk - KPC)
                        off = 128 * j if j > 0 else 0
                        sps = []
                        for h in range(hpc):
                            sp = ps_sp.tile([128, CH], F32, tag="sp",
                                            name=f"sp{h}")
                            nc.tensor.matmul(
                                sp[:, off:],
                                kT[:, h, kt * 128:(kt + 1) * 128],
                                qT[:, h, g * CH + off:(g + 1) * CH],
                                start=True, stop=True)
                            sps.append(sp)
                        # proj filler pairs for this step (delayed a few
                        # steps so ctxTn(g-1)'s 1/den DMA chain can land)
                        kd, nd = kt - 2, nk - 2
                        while gfi < len(gfill) and kd >= 0 and gfi + 1 <= (
                                len(gfill) * (kd + 1) + nd - 1) // nd:
                            st, oc = gfill[gfi]
                            emit_proj_tile(ctxTn, st, oc, s0)
                            gfi += 1
                        for h in range(hpc):
                            sp = sps[h]
                            if j >= 0:
                                # mask col c: masked iff c < p (strict tri);
                                # only the first 128 cols of the suffix can hit
                                nc.vector.tensor_add(sp[:, off:off + 128],
                                                     sp[:, off:off + 128],
                                                     mask_sb[:, :])
                            ex = expp.tile([128, CH], BF16, tag="ex")
                            nc.scalar.activation(ex[:, off:], sp[:, off:],
                                                 AF.Exp)
                            nc.tensor.matmul(
                                avp[h][:, off:],
                                v_sb[:, kt, h * DH:(h + 1) * DH],
                                ex[:, off:],
                                start=(kt == 0), stop=(kt == nk - 1),
                                skip_group_check=True)
                            nc.tensor.matmul(
                                dnp[h][:, off:], ones_sb[:], ex[:, off:],
                                start=(kt == 0), stop=(kt == nk - 1),
                                skip_group_check=True)
                    # ---- per-head: evict ctx, build 1/den row, normalize ----
                    for h in range(hpc):
                        ctxu = ctxup.tile([128, CH], BF16, tag=f"ctxu{h}")
                        nc.scalar.copy(ctxu[:], avp[h][:])  # frees av bank
                        den_ch = denp.tile([1, CH], F32, tag=f"den_ch{h}")
                        nc.scalar.copy(den_ch[:], dnp[h][:])  # frees dn bank
                        # DRAM round-trip: row -> [128,4] -> recip -> row
                        den_d = dramp.tile([1, CH], F32, tag="den_d")
                        nc.sync.dma_start(den_d[:], den_ch[:])
                        den_t = denp.tile([128, KPC], F32, tag="den_t")
                        nc.sync.dma_start(
                            den_t[:],
                            den_d[:].rearrange("p (j q) -> (p q) j", j=KPC))
                        rden_t = denp.tile([128, KPC], F32, tag="rden_t")
                        nc.vector.reciprocal(rden_t[:], den_t[:])
                        rd_d = dramp.tile([1, CH], F32, tag="rd_d")
                        nc.sync.dma_start(
                            rd_d[:].rearrange("p (j q) -> (p q) j", j=KPC),
                            rden_t[:])
                        rden_row = denp.tile([1, CH], F32, tag="rden_row")
                        nc.sync.dma_start(rden_row[:], rd_d[:])
                        rdenb = denp.tile([128, CH], F32, tag="rdenb")
                        nc.gpsimd.partition_broadcast(rdenb[:], rden_row[:])
                        nc.vector.tensor_tensor(
                            ctxTn[:, h, g * CH:(g + 1) * CH], ctxu[:],
                            rdenb[:], op=ALU.mult)
                # last chunk's proj becomes filler for the next batch
                for st in range((NCH - 1) * KPC, NCH * KPC):
                    for oc in range(NOC):
                        pending.append((ctxTn, st, s0, oc))
            # tail: final batch's last-chunk proj
            for pctxn, st, ps0, oc in pending:
                emit_proj_tile(pctxn, st, oc, ps0)
    nc.finalize()
    return nc


def host_consts(CH=512):
    p = np.arange(128)[:, None]
    c = np.arange(128)[None, :]
    masks = np.where(c < p, np.float32(NEG), np.float32(0.0))
    return {
        "masks": np.ascontiguousarray(masks.astype(ml_dtypes.bfloat16)),
        "ones": np.ones((128, 1), dtype=ml_dtypes.bfloat16),
    }


def host_inputs(x, Wq, Wk, Wv, Wo, B=B_FULL, S=S_FULL, E=EMB, hpc=HPC,
                DH=HEAD_DIM, CH=512):
    """Shard + lay out the full inputs for the 8 cores (bf16)."""
    SB = B * S
    DHC = hpc * DH
    xT = np.ascontiguousarray(x.reshape(SB, E).T.astype(ml_dtypes.bfloat16))
    consts = host_consts(CH)

    in_maps = []
    for c in range(N_CORES):
        lo, hi = c * DHC, (c + 1) * DHC
        in_maps.append({
            "xT": xT,
            "wqT": np.ascontiguousarray(Wq[lo:hi, :].T.astype(ml_dtypes.bfloat16)),
            "wkT": np.ascontiguousarray(Wk[lo:hi, :].T.astype(ml_dtypes.bfloat16)),
            "wvT": np.ascontiguousarray(Wv[lo:hi, :].T.astype(ml_dtypes.bfloat16)),
            "woT": np.ascontiguousarray(Wo[:, lo:hi].T.astype(ml_dtypes.bfloat16)),
            **consts,
        })
    return in_maps


def kernel(x, Wq, Wk, Wv, Wo, bo):
    x = np.asarray(x, dtype=np.float32)
    Wq = np.asarray(Wq, dtype=np.float32)
    Wk = np.asarray(Wk, dtype=np.float32)
    Wv = np.asarray(Wv, dtype=np.float32)
    Wo = np.asarray(Wo, dtype=np.float32)
    bo = np.asarray(bo, dtype=np.float32)

    nc = build()
    in_maps = host_inputs(x, Wq, Wk, Wv, Wo)
    res = run_bass_kernel_spmd(nc, in_maps, list(range(N_CORES)))
    y = res.results[0]["y"].astype(np.float64)
    for c in range(1, N_CORES):
        y += res.results[c]["y"].astype(np.float64)
    y = (y + bo).astype(np.float32)
    return y.reshape(B_FULL, S_FULL, EMB)



# revision 4
# speedup vs baseline: 1.2008x; 1.2008x over previous
"""Trainium2 Bass kernel: cached causal self-attention (dense transformer block).

Full module: y = CausalAttn(x; Wq, Wk, Wv) @ Wo.T + bo with
  B=4, S=2048, E=2048, H=16 heads, Dh=128, fp32 inputs.

Distribution: 8-way tensor parallel over heads (2 heads per NeuronCore).
Each core computes Q/K/V projections for its 2 heads, causal-softmax
attention, and a partial output projection; the host sums the 8 partials
and adds the bias.

All matmul operands are bf16 (PSUM accumulation stays fp32): same PE
streaming rate as float32r but Fast-Weight-Load halves the LDWEIGHTS
cost, SBUF/DMA traffic halves, and power throttling drops.  End-to-end
rel err ~6e-3, well inside the 2e-2 gate.

Layout: x pre-transposed on host (xT [E, B*S]); scores computed
transposed (sT[k, q]) so exp(sT) feeds attn@V directly with no on-chip
transpose.

Softmax denominators (this version): instead of a per-k-tile ones-vector
matmul (which costs the PE a third of the attention streaming), the exp
tiles are accumulated on the DVE into exsum[k, 2, q]; at chunk end ONE
all-ones [128,128] matmul per head partition-reduces exsum giving the
denominator already broadcast across all 128 partitions, so a direct
DVE reciprocal yields 1/den with no DRAM round-trip and the output
projection unblocks within ~2us of the last attention step.

Per k-step the PE runs s0,s1 (one 2-bank PSUM tile [128,2,512]),
av0,av1 and proj filler matmuls while ACT runs ONE merged exp of the
previous step's scores and the DVE accumulates exsum.  The output
projection accumulates BOTH heads into one PSUM bank (ctx is
pre-normalized by 1/den), and its matmul pairs are spread through the
attention steps and the next batch's QKV phase as PE filler work.
"""

import math

import ml_dtypes
import numpy as np

import concourse.bacc as bacc
import concourse.mybir as mybir
import concourse.tile as tile
from concourse.bass_utils import run_bass_kernel_spmd

F32 = mybir.dt.float32
BF16 = mybir.dt.bfloat16
AF = mybir.ActivationFunctionType
ALU = mybir.AluOpType

NEG = -1.0e30

# Full-problem constants
EMB = 2048
N_HEADS = 16
HEAD_DIM = 128
B_FULL = 4
S_FULL = 2048
N_CORES = 8
HPC = N_HEADS // N_CORES  # heads per core = 2


def build(B=B_FULL, S=S_FULL, E=EMB, hpc=HPC, DH=HEAD_DIM, CH=512):
    """Build the per-core Bass program (same program on all 8 cores)."""
    assert hpc == 2
    SB = B * S
    DHC = hpc * DH          # per-core head dims (256)
    NE = E // 128           # e-tiles (contraction tiles)
    NEH = NE // 2
    NCH = S // CH           # 512-wide chunks per sequence
    KPC = CH // 128         # k-tiles per chunk (4)
    NST = S // 128          # 128-row s-tiles per sequence
    NOC = E // CH           # output chunks
    scale = 1.0 / math.sqrt(DH)

    nc = bacc.Bacc("TRN2", target_bir_lowering=False, debug=False,
                   num_devices=N_CORES)

    xT = nc.dram_tensor("xT", [E, SB], BF16, kind="ExternalInput")
    wqT = nc.dram_tensor("wqT", [E, DHC], BF16, kind="ExternalInput")
    wkT = nc.dram_tensor("wkT", [E, DHC], BF16, kind="ExternalInput")
    wvT = nc.dram_tensor("wvT", [E, DHC], BF16, kind="ExternalInput")
    woT = nc.dram_tensor("woT", [DHC, E], BF16, kind="ExternalInput")
    masks = nc.dram_tensor("masks", [128, 128], BF16, kind="ExternalInput")
    ones = nc.dram_tensor("ones", [128, 128], BF16, kind="ExternalInput")
    y = nc.dram_tensor("y", [SB, E], BF16, kind="ExternalOutput")

    with tile.TileContext(nc) as tc:
        with (
            tc.tile_pool(name="wpool", bufs=1) as wpool,
            tc.tile_pool(name="xtp", bufs=2) as xtp,
            tc.tile_pool(name="qpool", bufs=1) as qpool,
            tc.tile_pool(name="kvpool", bufs=2) as kvpool,
            tc.tile_pool(name="ctxup", bufs=2) as ctxup,
            tc.tile_pool(name="expp", bufs=4) as expp,
            tc.tile_pool(name="esp", bufs=2) as esp,
            tc.tile_pool(name="denp", bufs=2) as denp,
            tc.tile_pool(name="yp", bufs=4) as yp,
            tc.tile_pool(name="ps_sp", bufs=2, space="PSUM") as ps_sp,
            tc.tile_pool(name="ps_av", bufs=1, space="PSUM") as ps_av,
            tc.tile_pool(name="ps_pj", bufs=2, space="PSUM") as ps_pj,
        ):
            # Resident weights / constants
            wq_sb = wpool.tile([128, NE, DHC], BF16, tag="wq")
            wk_sb = wpool.tile([128, NE, DHC], BF16, tag="wk")
            wv_sb = wpool.tile([128, NE, DHC], BF16, tag="wv")
            wo_sb = wpool.tile([128, hpc, E], BF16, tag="wo")
            xT_r = xT.rearrange("(t p) s -> p t s", p=128)
            wq_r = wqT.rearrange("(t p) d -> p t d", p=128)
            wk_r = wkT.rearrange("(t p) d -> p t d", p=128)
            # interleave weight halves with the first x chunk so the first
            # Q accumulation (wq + x) starts as early as possible
            x0a = xtp.tile([128, NEH, CH], BF16, tag="xta", name="x0a")
            x0b = xtp.tile([128, NEH, CH], BF16, tag="xtb", name="x0b")
            nc.sync.dma_start(wq_sb[:, 0:NEH, :], wq_r[:, 0:NEH, :])
            nc.sync.dma_start(x0a[:], xT_r[:, 0:NEH, 0:CH])
            nc.sync.dma_start(wq_sb[:, NEH:NE, :], wq_r[:, NEH:NE, :])
            nc.sync.dma_start(x0b[:], xT_r[:, NEH:NE, 0:CH])
            nc.sync.dma_start(wk_sb[:, 0:NEH, :], wk_r[:, 0:NEH, :])
            nc.sync.dma_start(wk_sb[:, NEH:NE, :], wk_r[:, NEH:NE, :])
            xpre = ((0, 0), x0a, x0b)
            nc.sync.dma_start(wv_sb[:], wvT.rearrange("(t p) d -> p t d", p=128))
            nc.sync.dma_start(wo_sb[:], woT.rearrange("(h p) e -> p h e", p=128))
            mask_sb = wpool.tile([128, 128], BF16, tag="mask")
            nc.sync.dma_start(mask_sb[:], masks[:, :])
            ones_sb = wpool.tile([128, 128], BF16, tag="ones")
            nc.sync.dma_start(ones_sb[:], ones[:, :])

            evict_parity = [0]

            def emit_proj_tile(pctxn, st, oc, ps0, phase="A"):
                """One output tile [128 q, CH]: both heads accumulated into one
                PSUM bank, plain-copy evict, y DMA.  During attention (phase B)
                the ACT engine is exp-bound, so evicts go to the DVE; in the
                QKV phase they alternate ACT/DVE."""
                p = ps_pj.tile([128, CH], F32, tag="pj")
                o0 = oc * CH
                nc.tensor.matmul(p[:], pctxn[:, 0, st * 128:(st + 1) * 128],
                                 wo_sb[:, 0, o0:o0 + CH], start=True, stop=False)
                nc.tensor.matmul(p[:], pctxn[:, 1, st * 128:(st + 1) * 128],
                                 wo_sb[:, 1, o0:o0 + CH], start=False, stop=True)
                ysb = yp.tile([128, CH], BF16, tag="ysb")
                if phase == "A" and evict_parity[0] % 2 == 0:
                    nc.scalar.copy(ysb[:], p[:])
                else:
                    nc.vector.tensor_copy(ysb[:], p[:])
                evict_parity[0] += 1
                nc.gpsimd.dma_start(
                    y[ps0 + st * 128:ps0 + (st + 1) * 128, o0:o0 + CH], ysb[:])

            # pending proj work from the previous batch's last chunk:
            # list of (ctxn_tile, st, ps0) emitted as filler during phase A
            pending = []

            for b in range(B):
                s0 = b * S
                qT = qpool.tile([128, hpc, S], BF16, tag="qT")
                ctxTn = qpool.tile([128, hpc, S], BF16, tag="ctxn")
                kT = kvpool.tile([128, hpc, S], BF16, tag="kT")
                v_sb = kvpool.tile([128, NST, DHC], BF16, tag="v")

                # ---------------- Phase A: Q/K/V projections -------------
                fillers = list(pending)
                pending = []
                fi = 0
                n_groups = NCH * (2 * hpc + KPC)
                gi = 0

                def maybe_fill_a():
                    nonlocal fi, gi
                    gi += 1
                    gd, nd = gi - 2, n_groups - 2
                    while fi < len(fillers) and gd >= 1 and fi + 1 <= (
                            len(fillers) * gd + nd - 1) // nd:
                        pctxn, st, ps0, oc = fillers[fi]
                        emit_proj_tile(pctxn, st, oc, ps0)
                        fi += 1

                for ch in range(NCH):
                    c0 = ch * CH
                    if xpre is not None and xpre[0] == (b, ch):
                        xta, xtb = xpre[1], xpre[2]
                    else:
                        xta = xtp.tile([128, NEH, CH], BF16, tag="xta")
                        nc.sync.dma_start(xta[:],
                                          xT_r[:, 0:NEH, s0 + c0:s0 + c0 + CH])
                        xtb = xtp.tile([128, NEH, CH], BF16, tag="xtb")
                        nc.sync.dma_start(xtb[:],
                                          xT_r[:, NEH:NE, s0 + c0:s0 + c0 + CH])
                    if ch + 1 < NCH or b + 1 < B:
                        nb_, nch = (b, ch + 1) if ch + 1 < NCH else (b + 1, 0)
                        n0 = nb_ * S + nch * CH
                        xna = xtp.tile([128, NEH, CH], BF16, tag="xta",
                                       name="xna")
                        nc.sync.dma_start(xna[:], xT_r[:, 0:NEH, n0:n0 + CH])
                        xnb = xtp.tile([128, NEH, CH], BF16, tag="xtb",
                                       name="xnb")
                        nc.sync.dma_start(xnb[:], xT_r[:, NEH:NE, n0:n0 + CH])
                        xpre = ((nb_, nch), xna, xnb)
                    else:
                        xpre = None

                    def xslice(et, lo=None, hi=None):
                        t = xta if et < NEH else xtb
                        e = et if et < NEH else et - NEH
                        if lo is None:
                            return t[:, e, :]
                        return t[:, e, lo:hi]

                    for h in range(hpc):
                        qp = ps_pj.tile([128, CH], F32, tag="pj")
                        for et in range(NE):
                            nc.tensor.matmul(
                                qp[:], wq_sb[:, et, h * DH:(h + 1) * DH],
                                xslice(et),
                                start=(et == 0), stop=(et == NE - 1))
                        nc.scalar.activation(qT[:, h, c0:c0 + CH], qp[:],
                                             AF.Identity, scale=scale)
                        maybe_fill_a()
                        kp = ps_pj.tile([128, CH], F32, tag="pj")
                        for et in range(NE):
                            nc.tensor.matmul(
                                kp[:], wk_sb[:, et, h * DH:(h + 1) * DH],
                                xslice(et),
                                start=(et == 0), stop=(et == NE - 1))
                        nc.scalar.activation(kT[:, h, c0:c0 + CH], kp[:],
                                             AF.Identity)
                        maybe_fill_a()
                    for st in range(KPC):
                        vp = ps_pj.tile([128, DHC], F32, tag="pj")
                        for et in range(NE):
                            nc.tensor.matmul(
                                vp[:], xslice(et, st * 128, (st + 1) * 128),
                                wv_sb[:, et, :],
                                start=(et == 0), stop=(et == NE - 1))
                        nc.scalar.activation(v_sb[:, ch * KPC + st, :], vp[:],
                                             AF.Identity)
                        maybe_fill_a()
                # any leftover fillers
                while fi < len(fillers):
                    pctxn, st, ps0, oc = fillers[fi]
                    emit_proj_tile(pctxn, st, oc, ps0)
                    fi += 1

                # ------- Phase B: attention, one-step software pipeline -----
                # Per step the PE runs s(kt) then av(kt-1); the merged exp(kt)
                # on ACT hides under s(kt+1) + av(kt) + fillers.  Proj filler
                # emission is rate-capped (2 tiles per 3 steps; the last batch
                # 4 per 3) -- overflow tiles spill into the next batch's QKV
                # phase where the PE has ACT/DVE headroom to spare.
                bfill = []      # proj tiles available for in-B emission
                emitted = [0]
                budget = [0]
                rnum, rden_ = (4, 3) if b == B - 1 else (2, 3)

                def maybe_fill_b():
                    budget[0] += rnum
                    while bfill and emitted[0] + 1 <= budget[0] // rden_:
                        st, oc = bfill.pop(0)
                        emit_proj_tile(ctxTn, st, oc, s0, phase="B")
                        emitted[0] += 1

                for g in range(NCH):
                    nk = KPC * (g + 1)
                    if g > 0:
                        for st in range((g - 1) * KPC, g * KPC):
                            for oc in range(NOC):
                                bfill.append((st, oc))
                    avp = ps_av.tile([128, hpc, CH], F32, tag="av")
                    exsum = esp.tile([128, hpc, CH], F32, tag="exsum")
                    prev = None
                    for kt in range(nk):
                        j = kt - (nk - KPC)
                        off = 128 * j if j > 0 else 0
                        sp = ps_sp.tile([128, hpc, CH], F32, tag="sp")
                        for h in range(hpc):
                            nc.tensor.matmul(
                                sp[:, h, off:],
                                kT[:, h, kt * 128:(kt + 1) * 128],
                                qT[:, h, g * CH + off:(g + 1) * CH],
                                start=True, stop=True)
                        if j >= 0:
                            # mask col c: masked iff c < p (strict tri);
                            # only the first 128 cols of the suffix can hit
                            for h in range(hpc):
                                nc.vector.tensor_add(sp[:, h, off:off + 128],
                                                     sp[:, h, off:off + 128],
                                                     mask_sb[:, :])
                        ex = expp.tile([128, hpc, CH], BF16, tag="ex")
                        nc.scalar.activation(ex[:, :, off:], sp[:, :, off:],
                                             AF.Exp)
                        # delayed by >=2 steps so chunk g-1's den/normalize
                        # can land before its proj tiles hit the PE queue
                        if kt >= 2:
                            maybe_fill_b()
                        if prev is not None:
                            pex, poff, pkt = prev
                            for h in range(hpc):
                                nc.tensor.matmul(
                                    avp[:, h, poff:],
                                    v_sb[:, pkt, h * DH:(h + 1) * DH],
                                    pex[:, h, poff:],
                                    start=(pkt == 0), stop=(pkt == nk - 1),
                                    skip_group_check=True)
                            if pkt == 0:
                                nc.vector.tensor_copy(exsum[:], pex[:])
                            else:
                                nc.vector.tensor_add(exsum[:, :, poff:],
                                                     exsum[:, :, poff:],
                                                     pex[:, :, poff:])
                        prev = (ex, off, kt)
                    # drain the lagged last step
                    pex, poff, pkt = prev
                    for h in range(hpc):
                        nc.tensor.matmul(
                            avp[:, h, poff:],
                            v_sb[:, pkt, h * DH:(h + 1) * DH],
                            pex[:, h, poff:],
                            start=(pkt == 0), stop=(pkt == nk - 1),
                            skip_group_check=True)
                    nc.vector.tensor_add(exsum[:, :, poff:],
                                         exsum[:, :, poff:],
                                         pex[:, :, poff:])
                    # ---- chunk end: den via one all-ones matmul per head,
                    # direct reciprocal, evict + normalize ctx ----
                    exsum_b = esp.tile([128, hpc, CH], BF16, tag="exsum_b")
                    nc.vector.tensor_copy(exsum_b[:], exsum[:])
                    dnp = ps_sp.tile([128, hpc, CH], F32, tag="sp", name="dnp")
                    for h in range(hpc):
                        nc.tensor.matmul(dnp[:, h, :], ones_sb[:],
                                         exsum_b[:, h, :],
                                         start=True, stop=True)
                    rdenb = denp.tile([128, hpc, CH], F32, tag="rdenb")
                    nc.vector.reciprocal(rdenb[:], dnp[:])
                    ctxu = ctxup.tile([128, hpc, CH], BF16, tag="ctxu")
                    nc.scalar.copy(ctxu[:], avp[:])  # frees av banks
                    nc.vector.tensor_tensor(
                        ctxTn[:, :, g * CH:(g + 1) * CH], ctxu[:],
                        rdenb[:], op=ALU.mult)
                # unemitted + last chunk's proj become next-batch fillers
                for st, oc in bfill:
                    pending.append((ctxTn, st, s0, oc))
                for st in range((NCH - 1) * KPC, NCH * KPC):
                    for oc in range(NOC):
                        pending.append((ctxTn, st, s0, oc))
            # tail: final batch's last-chunk proj
            for pctxn, st, ps0, oc in pending:
                emit_proj_tile(pctxn, st, oc, ps0)
    nc.finalize()
    return nc


def host_consts(CH=512):
    p = np.arange(128)[:, None]
    c = np.arange(128)[None, :]
    masks = np.where(c < p, np.float32(NEG), np.float32(0.0))
    return {
        "masks": np.ascontiguousarray(masks.astype(ml_dtypes.bfloat16)),
        "ones": np.ones((128, 128), dtype=ml_dtypes.bfloat16),
    }


def host_inputs(x, Wq, Wk, Wv, Wo, B=B_FULL, S=S_FULL, E=EMB, hpc=HPC,
                DH=HEAD_DIM, CH=512):
    """Shard + lay out the full inputs for the 8 cores (bf16)."""
    SB = B * S
    DHC = hpc * DH
    xT = np.ascontiguousarray(x.reshape(SB, E).T.astype(ml_dtypes.bfloat16))
    consts = host_consts(CH)

    in_maps = []
    for c in range(N_CORES):
        lo, hi = c * DHC, (c + 1) * DHC
        in_maps.append({
            "xT": xT,
            "wqT": np.ascontiguousarray(Wq[lo:hi, :].T.astype(ml_dtypes.bfloat16)),
            "wkT": np.ascontiguousarray(Wk[lo:hi, :].T.astype(ml_dtypes.bfloat16)),
            "wvT": np.ascontiguousarray(Wv[lo:hi, :].T.astype(ml_dtypes.bfloat16)),
            "woT": np.ascontiguousarray(Wo[:, lo:hi].T.astype(ml_dtypes.bfloat16)),
            **consts,
        })
    return in_maps


def kernel(x, Wq, Wk, Wv, Wo, bo):
    x = np.asarray(x, dtype=np.float32)
    Wq = np.asarray(Wq, dtype=np.float32)
    Wk = np.asarray(Wk, dtype=np.float32)
    Wv = np.asarray(Wv, dtype=np.float32)
    Wo = np.asarray(Wo, dtype=np.float32)
    bo = np.asarray(bo, dtype=np.float32)

    nc = build()
    in_maps = host_inputs(x, Wq, Wk, Wv, Wo)
    res = run_bass_kernel_spmd(nc, in_maps, list(range(N_CORES)))
    y = res.results[0]["y"].astype(np.float64)
    for c in range(1, N_CORES):
        y += res.results[c]["y"].astype(np.float64)
    y = (y + bo).astype(np.float32)
    return y.reshape(B_FULL, S_FULL, EMB)


# revision 6
# speedup vs baseline: 1.3866x; 1.1548x over previous
"""Trainium2 Bass kernel: cached causal self-attention (dense transformer block).

Full module: y = CausalAttn(x; Wq, Wk, Wv) @ Wo.T + bo with
  B=4, S=2048, E=2048, H=16 heads, Dh=128, fp32 inputs.

Distribution: 8-way tensor parallel over heads (2 heads per NeuronCore).
Each core computes Q/K/V projections for its 2 heads, causal-softmax
attention, and a partial output projection; the host sums the 8 partials
and adds the bias.

All matmul operands are bf16 (PSUM accumulation stays fp32): same PE
streaming rate as float32r but Fast-Weight-Load halves the LDWEIGHTS
cost, SBUF/DMA traffic halves, and power throttling drops.  End-to-end
rel err ~6e-3, well inside the 2e-2 gate.

Layout: x pre-transposed on host (xT [E, B*S]); scores computed
transposed (sT[k, q]) so exp(sT) feeds attn@V directly with no on-chip
transpose.

Softmax denominators (this version): instead of a per-k-tile ones-vector
matmul (which costs the PE a third of the attention streaming), the exp
tiles are accumulated on the DVE into exsum[k, 2, q]; at chunk end ONE
all-ones [128,128] matmul per head partition-reduces exsum giving the
denominator already broadcast across all 128 partitions, so a direct
DVE reciprocal yields 1/den with no DRAM round-trip and the output
projection unblocks within ~2us of the last attention step.

Per k-step the PE runs s0,s1 (one 2-bank PSUM tile [128,2,512]),
av0,av1 and proj filler matmuls while ACT runs ONE merged exp of the
previous step's scores and the DVE accumulates exsum.  The output
projection accumulates BOTH heads into one PSUM bank (ctx is
pre-normalized by 1/den), and its matmul pairs are spread through the
attention steps and the next batch's QKV phase as PE filler work.
"""

import math

import ml_dtypes
import numpy as np

import concourse.bacc as bacc
import concourse.mybir as mybir
import concourse.tile as tile
from concourse.bass_utils import run_bass_kernel_spmd

F32 = mybir.dt.float32
BF16 = mybir.dt.bfloat16
F16 = mybir.dt.float16
AF = mybir.ActivationFunctionType
ALU = mybir.AluOpType

NEG = -1.0e30
# exp() output pre-scale 2^-6 (bias folded into the activation) keeps the
# fp16 exp-sum accumulators far from overflow; 1/den absorbs it exactly.
EXP_BIAS = -6.0 * math.log(2.0)

# Full-problem constants
EMB = 2048
N_HEADS = 16
HEAD_DIM = 128
B_FULL = 4
S_FULL = 2048
N_CORES = 8
HPC = N_HEADS // N_CORES  # heads per core = 2


def build(B=B_FULL, S=S_FULL, E=EMB, hpc=HPC, DH=HEAD_DIM, CH=512):
    """Build the per-core Bass program (same program on all 8 cores)."""
    assert hpc == 2
    SB = B * S
    DHC = hpc * DH          # per-core head dims (256)
    NE = E // 128           # e-tiles (contraction tiles)
    NEH = NE // 2
    NCH = S // CH           # 512-wide chunks per sequence
    KPC = CH // 128         # k-tiles per chunk (4)
    NST = S // 128          # 128-row s-tiles per sequence
    NOC = E // CH           # output chunks
    scale = 1.0 / math.sqrt(DH)

    nc = bacc.Bacc("TRN2", target_bir_lowering=False, debug=False,
                   num_devices=N_CORES)

    xT = nc.dram_tensor("xT", [E, SB], BF16, kind="ExternalInput")
    wqT = nc.dram_tensor("wqT", [E, DHC], BF16, kind="ExternalInput")
    wkT = nc.dram_tensor("wkT", [E, DHC], BF16, kind="ExternalInput")
    wvT = nc.dram_tensor("wvT", [E, DHC], BF16, kind="ExternalInput")
    woT = nc.dram_tensor("woT", [DHC, E], BF16, kind="ExternalInput")
    masks = nc.dram_tensor("masks", [128, 2, 128], BF16, kind="ExternalInput")
    ones = nc.dram_tensor("ones", [128, 128], F16, kind="ExternalInput")
    ebias = nc.dram_tensor("ebias", [128, 1], F32, kind="ExternalInput")
    y = nc.dram_tensor("y", [SB, E], BF16, kind="ExternalOutput")

    with tile.TileContext(nc) as tc:
        with (
            tc.tile_pool(name="wpool", bufs=1) as wpool,
            tc.tile_pool(name="xtp", bufs=2) as xtp,
            tc.tile_pool(name="qpool", bufs=1) as qpool,
            tc.tile_pool(name="kvpool", bufs=2) as kvpool,
            tc.tile_pool(name="expp", bufs=4) as expp,
            tc.tile_pool(name="esp", bufs=2) as esp,
            tc.tile_pool(name="denp", bufs=2) as denp,
            tc.tile_pool(name="yp", bufs=4) as yp,
            tc.tile_pool(name="ps_sp", bufs=2, space="PSUM") as ps_sp,
            tc.tile_pool(name="ps_av", bufs=1, space="PSUM") as ps_av,
            tc.tile_pool(name="ps_pj", bufs=2, space="PSUM") as ps_pj,
        ):
            # Resident weights / constants
            wq_sb = wpool.tile([128, NE, DHC], BF16, tag="wq")
            wk_sb = wpool.tile([128, NE, DHC], BF16, tag="wk")
            wv_sb = wpool.tile([128, NE, DHC], BF16, tag="wv")
            wo_sb = wpool.tile([128, hpc, E], BF16, tag="wo")
            xT_r = xT.rearrange("(t p) s -> p t s", p=128)
            wq_r = wqT.rearrange("(t p) d -> p t d", p=128)
            wk_r = wkT.rearrange("(t p) d -> p t d", p=128)
            # interleave weight halves with the first x chunk so the first
            # Q accumulation (wq + x) starts as early as possible
            x0a = xtp.tile([128, NEH, CH], BF16, tag="xta", name="x0a")
            x0b = xtp.tile([128, NEH, CH], BF16, tag="xtb", name="x0b")
            nc.sync.dma_start(wq_sb[:, 0:NEH, :], wq_r[:, 0:NEH, :])
            nc.sync.dma_start(x0a[:], xT_r[:, 0:NEH, 0:CH])
            nc.sync.dma_start(wq_sb[:, NEH:NE, :], wq_r[:, NEH:NE, :])
            nc.sync.dma_start(x0b[:], xT_r[:, NEH:NE, 0:CH])
            nc.sync.dma_start(wk_sb[:, 0:NEH, :], wk_r[:, 0:NEH, :])
            nc.sync.dma_start(wk_sb[:, NEH:NE, :], wk_r[:, NEH:NE, :])
            xpre = ((0, 0), x0a, x0b)
            nc.sync.dma_start(wv_sb[:], wvT.rearrange("(t p) d -> p t d", p=128))
            nc.sync.dma_start(wo_sb[:], woT.rearrange("(h p) e -> p h e", p=128))
            mask_sb = wpool.tile([128, 2, 128], BF16, tag="mask")
            nc.sync.dma_start(mask_sb[:], masks[:, :, :])
            ones_sb = wpool.tile([128, 128], F16, tag="ones")
            nc.sync.dma_start(ones_sb[:], ones[:, :])
            ebias_sb = wpool.tile([128, 1], F32, tag="ebias")
            nc.sync.dma_start(ebias_sb[:], ebias[:, :])

            evict_parity = [0]

            def emit_proj_tile(pctxn, st, oc, ps0, phase="A"):
                """One output tile [128 q, CH]: both heads accumulated into one
                PSUM bank, plain-copy evict, y DMA.  During attention (phase B)
                the ACT engine is exp-bound, so evicts go to the DVE; in the
                QKV phase they alternate ACT/DVE."""
                p = ps_pj.tile([128, CH], F32, tag="pj")
                o0 = oc * CH
                nc.tensor.matmul(p[:], pctxn[:, 0, st * 128:(st + 1) * 128],
                                 wo_sb[:, 0, o0:o0 + CH], start=True, stop=False)
                nc.tensor.matmul(p[:], pctxn[:, 1, st * 128:(st + 1) * 128],
                                 wo_sb[:, 1, o0:o0 + CH], start=False, stop=True)
                ysb = yp.tile([128, CH], BF16, tag="ysb")
                if (phase == "A" and evict_parity[0] % 2 == 0) or (
                        phase == "B" and evict_parity[0] % 4 == 0):
                    nc.scalar.copy(ysb[:], p[:])
                else:
                    nc.vector.tensor_copy(ysb[:], p[:])
                evict_parity[0] += 1
                nc.gpsimd.dma_start(
                    y[ps0 + st * 128:ps0 + (st + 1) * 128, o0:o0 + CH], ysb[:])

            # pending proj work from the previous batch's last chunk:
            # list of (ctxn_tile, st, ps0) emitted as filler during phase A
            pending = []

            for b in range(B):
                s0 = b * S
                qT = qpool.tile([128, hpc, S], BF16, tag="qT")
                ctxTn = qpool.tile([128, hpc, S], BF16, tag="ctxn")
                kT = kvpool.tile([128, hpc, S], BF16, tag="kT")
                v_sb = kvpool.tile([128, NST, DHC], F16, tag="v")

                # ---------------- Phase A: Q/K/V projections -------------
                fillers = list(pending)
                pending = []
                fi = 0
                n_groups = NCH * (2 * hpc + KPC)
                gi = 0

                def maybe_fill_a():
                    nonlocal fi, gi
                    gi += 1
                    gd, nd = gi - 2, n_groups - 2
                    while fi < len(fillers) and gd >= 1 and fi + 1 <= (
                            len(fillers) * gd + nd - 1) // nd:
                        pctxn, st, ps0, oc = fillers[fi]
                        emit_proj_tile(pctxn, st, oc, ps0)
                        fi += 1

                for ch in range(NCH):
                    c0 = ch * CH
                    if xpre is not None and xpre[0] == (b, ch):
                        xta, xtb = xpre[1], xpre[2]
                    else:
                        xta = xtp.tile([128, NEH, CH], BF16, tag="xta")
                        nc.sync.dma_start(xta[:],
                                          xT_r[:, 0:NEH, s0 + c0:s0 + c0 + CH])
                        xtb = xtp.tile([128, NEH, CH], BF16, tag="xtb")
                        nc.sync.dma_start(xtb[:],
                                          xT_r[:, NEH:NE, s0 + c0:s0 + c0 + CH])
                    if ch + 1 < NCH or b + 1 < B:
                        nb_, nch = (b, ch + 1) if ch + 1 < NCH else (b + 1, 0)
                        n0 = nb_ * S + nch * CH
                        xna = xtp.tile([128, NEH, CH], BF16, tag="xta",
                                       name="xna")
                        nc.sync.dma_start(xna[:], xT_r[:, 0:NEH, n0:n0 + CH])
                        xnb = xtp.tile([128, NEH, CH], BF16, tag="xtb",
                                       name="xnb")
                        nc.sync.dma_start(xnb[:], xT_r[:, NEH:NE, n0:n0 + CH])
                        xpre = ((nb_, nch), xna, xnb)
                    else:
                        xpre = None

                    def xslice(et, lo=None, hi=None):
                        t = xta if et < NEH else xtb
                        e = et if et < NEH else et - NEH
                        if lo is None:
                            return t[:, e, :]
                        return t[:, e, lo:hi]

                    for h in range(hpc):
                        qp = ps_pj.tile([128, CH], F32, tag="pj")
                        for et in range(NE):
                            nc.tensor.matmul(
                                qp[:], wq_sb[:, et, h * DH:(h + 1) * DH],
                                xslice(et),
                                start=(et == 0), stop=(et == NE - 1))
                        nc.scalar.activation(qT[:, h, c0:c0 + CH], qp[:],
                                             AF.Identity, scale=scale)
                        maybe_fill_a()
                        kp = ps_pj.tile([128, CH], F32, tag="pj")
                        for et in range(NE):
                            nc.tensor.matmul(
                                kp[:], wk_sb[:, et, h * DH:(h + 1) * DH],
                                xslice(et),
                                start=(et == 0), stop=(et == NE - 1))
                        nc.scalar.activation(kT[:, h, c0:c0 + CH], kp[:],
                                             AF.Identity)
                        maybe_fill_a()
                    for st in range(KPC):
                        vp = ps_pj.tile([128, DHC], F32, tag="pj")
                        for et in range(NE):
                            nc.tensor.matmul(
                                vp[:], xslice(et, st * 128, (st + 1) * 128),
                                wv_sb[:, et, :],
                                start=(et == 0), stop=(et == NE - 1))
                        nc.scalar.activation(v_sb[:, ch * KPC + st, :], vp[:],
                                             AF.Identity)
                        maybe_fill_a()
                # any leftover fillers
                while fi < len(fillers):
                    pctxn, st, ps0, oc = fillers[fi]
                    emit_proj_tile(pctxn, st, oc, ps0)
                    fi += 1

                # ------- Phase B: attention, one-step software pipeline -----
                # Per step the PE runs s(kt) then av(kt-1); the merged exp(kt)
                # on ACT hides under s(kt+1) + av(kt) + fillers.  Proj filler
                # emission is rate-capped (2 tiles per 3 steps; the last batch
                # 4 per 3) -- overflow tiles spill into the next batch's QKV
                # phase where the PE has ACT/DVE headroom to spare.
                bfill = []      # proj tiles available for in-B emission
                emitted = [0]
                budget = [0]
                rnum, rden_ = (4, 3) if b == B - 1 else (2, 3)

                def maybe_fill_b():
                    budget[0] += rnum
                    while bfill and emitted[0] + 1 <= budget[0] // rden_:
                        st, oc = bfill.pop(0)
                        emit_proj_tile(ctxTn, st, oc, s0, phase="B")
                        emitted[0] += 1

                for g in range(NCH):
                    nk = KPC * (g + 1)
                    if g > 0:
                        for st in range((g - 1) * KPC, g * KPC):
                            for oc in range(NOC):
                                bfill.append((st, oc))
                    avp = ps_av.tile([128, hpc, CH], F32, tag="av")
                    exsum = esp.tile([128, hpc, CH], F16, tag="exsum")
                    prev = None
                    for kt in range(nk):
                        j = kt - (nk - KPC)
                        off = 128 * j if j > 0 else 0
                        sp = ps_sp.tile([128, hpc, CH], F32, tag="sp")
                        for h in range(hpc):
                            nc.tensor.matmul(
                                sp[:, h, off:],
                                kT[:, h, kt * 128:(kt + 1) * 128],
                                qT[:, h, g * CH + off:(g + 1) * CH],
                                start=True, stop=True)
                        if j >= 0:
                            # mask col c: masked iff c < p (strict tri);
                            # only the first 128 cols of the suffix can hit
                            nc.vector.tensor_add(sp[:, :, off:off + 128],
                                                 sp[:, :, off:off + 128],
                                                 mask_sb[:, :, :])
                        ex = expp.tile([128, hpc, CH], F16, tag="ex")
                        nc.scalar.activation(ex[:, :, off:], sp[:, :, off:],
                                             AF.Exp, bias=ebias_sb[:])
                        # delayed by >=2 steps so chunk g-1's den/normalize
                        # can land before its proj tiles hit the PE queue
                        if kt >= 2:
                            maybe_fill_b()
                        if prev is not None:
                            pex, poff, pkt = prev
                            for h in range(hpc):
                                nc.tensor.matmul(
                                    avp[:, h, poff:],
                                    v_sb[:, pkt, h * DH:(h + 1) * DH],
                                    pex[:, h, poff:],
                                    start=(pkt == 0), stop=(pkt == nk - 1),
                                    skip_group_check=True)
                            if pkt == 0:
                                nc.vector.tensor_copy(exsum[:], pex[:])
                            else:
                                nc.vector.tensor_add(exsum[:, :, poff:],
                                                     exsum[:, :, poff:],
                                                     pex[:, :, poff:])
                        prev = (ex, off, kt)
                    # drain the lagged last step
                    pex, poff, pkt = prev
                    for h in range(hpc):
                        nc.tensor.matmul(
                            avp[:, h, poff:],
                            v_sb[:, pkt, h * DH:(h + 1) * DH],
                            pex[:, h, poff:],
                            start=(pkt == 0), stop=(pkt == nk - 1),
                            skip_group_check=True)
                    nc.vector.tensor_add(exsum[:, :, poff:],
                                         exsum[:, :, poff:],
                                         pex[:, :, poff:])
                    # ---- chunk end: den via one all-ones matmul per head,
                    # direct reciprocal, evict + normalize ctx ----
                    dnp = ps_sp.tile([128, hpc, CH], F32, tag="sp", name="dnp")
                    for h in range(hpc):
                        nc.tensor.matmul(dnp[:, h, :], ones_sb[:],
                                         exsum[:, h, :],
                                         start=True, stop=True)
                    rdenb = denp.tile([128, hpc, CH], F32, tag="rdenb")
                    nc.vector.reciprocal_approx_fast(rdenb[:], dnp[:])
                    # normalize straight out of the AV PSUM banks (frees them)
                    nc.vector.tensor_tensor(
                        ctxTn[:, :, g * CH:(g + 1) * CH], avp[:],
                        rdenb[:], op=ALU.mult)
                # unemitted + last chunk's proj become next-batch fillers
                for st, oc in bfill:
                    pending.append((ctxTn, st, s0, oc))
                for st in range((NCH - 1) * KPC, NCH * KPC):
                    for oc in range(NOC):
                        pending.append((ctxTn, st, s0, oc))
            # tail: final batch's last-chunk proj
            for pctxn, st, ps0, oc in pending:
                emit_proj_tile(pctxn, st, oc, ps0)
    nc.finalize()
    return nc


def host_consts(CH=512):
    p = np.arange(128)[:, None]
    c = np.arange(128)[None, :]
    masks = np.where(c < p, np.float32(NEG), np.float32(0.0))
    masks2 = np.broadcast_to(masks[:, None, :], (128, 2, 128))
    return {
        "masks": np.ascontiguousarray(masks2.astype(ml_dtypes.bfloat16)),
        "ones": np.ones((128, 128), dtype=np.float16),
        "ebias": np.full((128, 1), EXP_BIAS, dtype=np.float32),
    }


def host_inputs(x, Wq, Wk, Wv, Wo, B=B_FULL, S=S_FULL, E=EMB, hpc=HPC,
                DH=HEAD_DIM, CH=512):
    """Shard + lay out the full inputs for the 8 cores (bf16)."""
    SB = B * S
    DHC = hpc * DH
    xT = np.ascontiguousarray(x.reshape(SB, E).T.astype(ml_dtypes.bfloat16))
    consts = host_consts(CH)

    in_maps = []
    for c in range(N_CORES):
        lo, hi = c * DHC, (c + 1) * DHC
        in_maps.append({
            "xT": xT,
            "wqT": np.ascontiguousarray(Wq[lo:hi, :].T.astype(ml_dtypes.bfloat16)),
            "wkT": np.ascontiguousarray(Wk[lo:hi, :].T.astype(ml_dtypes.bfloat16)),
            "wvT": np.ascontiguousarray(Wv[lo:hi, :].T.astype(ml_dtypes.bfloat16)),
            "woT": np.ascontiguousarray(Wo[:, lo:hi].T.astype(ml_dtypes.bfloat16)),
            **consts,
        })
    return in_maps


def kernel(x, Wq, Wk, Wv, Wo, bo):
    x = np.asarray(x, dtype=np.float32)
    Wq = np.asarray(Wq, dtype=np.float32)
    Wk = np.asarray(Wk, dtype=np.float32)
    Wv = np.asarray(Wv, dtype=np.float32)
    Wo = np.asarray(Wo, dtype=np.float32)
    bo = np.asarray(bo, dtype=np.float32)

    nc = build()
    in_maps = host_inputs(x, Wq, Wk, Wv, Wo)
    res = run_bass_kernel_spmd(nc, in_maps, list(range(N_CORES)))
    y = res.results[0]["y"].astype(np.float64)
    for c in range(1, N_CORES):
        y += res.results[c]["y"].astype(np.float64)
    y = (y + bo).astype(np.float32)
    return y.reshape(B_FULL, S_FULL, EMB)
